# revision 1
# baseline (speedup 1.0000x reference)
import sys

sys.path.insert(0, "/opt/trn_rl_repo")

import numpy as np

# ---- hardcoded problem constants (nn_Decoder_12979391169054) ----
B, LAT, NF, CH, S, NC_STEPS = 16, 512, 64, 3, 128, 8
FH = 2 * NF                      # 128
KS = [3, 5, 9, 17, 33]
PADS = [1, 2, 4, 8, 16]
EPS = 1e-5
HW = S * S                       # 16384
N_CORES = 8
SPC = B // N_CORES               # samples per core = 2

SIZES = [NF * FH, FH, FH * FH, FH, FH * NF, NF, NF * NF, NF]
OFFS = np.cumsum([0] + SIZES)

_NCHUNK = 512
_NCH = HW // _NCHUNK             # 32 chunks


def _toeplitz(kern, pad):
    k = len(kern)
    T = np.zeros((S, S), np.float32)
    for i in range(k):
        off = i - pad
        for h in range(S):
            h2 = h + off
            if 0 <= h2 < S:
                T[h, h2] = kern[i]
    return T


def _sobel_mats():
    mats = []
    for k, pad in zip(KS, PADS):
        r = np.linspace(-1.0, 1.0, k)
        s = np.sin(r * (np.pi / 2)).astype(np.float32)
        c = np.cos(r * (np.pi / 2)).astype(np.float32)
        mats.append((_toeplitz(c, pad), _toeplitz(s, pad)))
    return mats


_SOB = _sobel_mats()


def _sin_sobel(x):
    # x: (B, 3, S, S) -> (B, 33, S, S), channel order gx1,gy1,...,gx5,gy5,x
    outs = []
    for TC, TS_ in _SOB:
        # gx kernel = c (rows) x s (cols); gy = s (rows) x c (cols)
        hc = np.einsum('Hh,bchw->bcHw', TC, x)
        hs = np.einsum('Hh,bchw->bcHw', TS_, x)
        gx = np.einsum('Ww,bchw->bchW', TS_, hc)
        gy = np.einsum('Ww,bchw->bchW', TC, hs)
        outs.append(gx)
        outs.append(gy)
    outs.append(x)
    return np.concatenate(outs, axis=1)


def _conv1x1(x, w, b):
    # x (B,I,S,S), w (O,I)
    bsz = x.shape[0]
    y = np.matmul(w[None], x.reshape(bsz, x.shape[1], HW))
    return (y + b[None, :, None]).reshape(bsz, w.shape[0], S, S)


def _inst_norm(x):
    m = x.mean(axis=(2, 3), keepdims=True)
    v = x.var(axis=(2, 3), keepdims=True)
    return (x - m) / np.sqrt(v + EPS)


def _dyna_weights(lat, dyna_w, dyna_b):
    p = lat @ dyna_w.T + dyna_b          # (B, F_PARAMS)
    bsz = lat.shape[0]

    def take(i, shape):
        return p[:, OFFS[i]:OFFS[i + 1]].reshape((bsz,) + shape)

    def nrm(w):
        return w / (np.linalg.norm(w, axis=-1, keepdims=True) + 1e-8)

    w1 = nrm(take(0, (FH, NF))); b1 = take(1, (FH,))
    w2 = nrm(take(2, (FH, FH))); b2 = take(3, (FH,))
    w3 = nrm(take(4, (NF, FH))); b3 = take(5, (NF,))
    ws = nrm(take(6, (NF, NF))); bs = take(7, (NF,))
    return w1, b1, w2, b2, w3, b3, ws, bs


def _dyna_numpy(x, W):
    # x (B, NF, S, S); reference fallback path
    w1, b1, w2, b2, w3, b3, ws, bs = W
    bsz = x.shape[0]
    xf = x.reshape(bsz, NF, HW)
    h = np.maximum(np.matmul(w1, xf) + b1[:, :, None], 0.0)
    h = np.maximum(np.matmul(w2, h) + b2[:, :, None], 0.0)
    y = np.matmul(w3, h) + b3[:, :, None]
    y = y + np.matmul(ws, xf) + bs[:, :, None]
    return y.reshape(bsz, NF, S, S)


# ---------------- Bass/TRN2 device path ----------------
_DEV = {"nc": None, "ok": True}


def _build_nc():
    import concourse.bass as bass
    import concourse.tile as tile
    from concourse import mybir

    f32 = mybir.dt.float32
    nc = bass.Bass()
    x_in = nc.declare_dram_parameter("x", [SPC * NF, HW], f32, isOutput=False)
    w1t = nc.declare_dram_parameter("w1t", [SPC, NF, FH], f32, isOutput=False)
    w2t = nc.declare_dram_parameter("w2t", [SPC, FH, FH], f32, isOutput=False)
    w3t = nc.declare_dram_parameter("w3t", [SPC, FH, NF], f32, isOutput=False)
    wst = nc.declare_dram_parameter("wst", [SPC, NF, NF], f32, isOutput=False)
    b1 = nc.declare_dram_parameter("b1", [SPC, FH, 1], f32, isOutput=False)
    b2 = nc.declare_dram_parameter("b2", [SPC, FH, 1], f32, isOutput=False)
    b3s = nc.declare_dram_parameter("b3s", [SPC, NF, 1], f32, isOutput=False)
    y_out = nc.declare_dram_parameter("y", [SPC * NF, HW], f32, isOutput=True)

    relu = mybir.ActivationFunctionType.Relu

    with tile.TileContext(nc) as tc:
        with (
            tc.tile_pool(name="wp", bufs=1) as wp,
            tc.tile_pool(name="xp", bufs=1) as xp,
            tc.tile_pool(name="work", bufs=4) as work,
            tc.tile_pool(name="yp", bufs=4) as yp,
            tc.tile_pool(name="ps", bufs=8, space="PSUM") as ps,
        ):
            X = xp.tile([SPC * NF, HW], f32, tag="X")
            nc.sync.dma_start(out=X, in_=x_in[:, :])
            Wt = []
            for s in range(SPC):
                t1 = wp.tile([NF, FH], f32, tag=f"w1_{s}")
                nc.sync.dma_start(out=t1, in_=w1t[s, :, :])
                t2 = wp.tile([FH, FH], f32, tag=f"w2_{s}")
                nc.sync.dma_start(out=t2, in_=w2t[s, :, :])
                t3 = wp.tile([FH, NF], f32, tag=f"w3_{s}")
                nc.sync.dma_start(out=t3, in_=w3t[s, :, :])
                t4 = wp.tile([NF, NF], f32, tag=f"ws_{s}")
                nc.sync.dma_start(out=t4, in_=wst[s, :, :])
                tb1 = wp.tile([FH, 1], f32, tag=f"b1_{s}")
                nc.sync.dma_start(out=tb1, in_=b1[s, :, :])
                tb2 = wp.tile([FH, 1], f32, tag=f"b2_{s}")
                nc.sync.dma_start(out=tb2, in_=b2[s, :, :])
                tb3 = wp.tile([NF, 1], f32, tag=f"b3_{s}")
                nc.sync.dma_start(out=tb3, in_=b3s[s, :, :])
                Wt.append((t1, t2, t3, t4, tb1, tb2, tb3))

            for ci in range(_NCH):
                sl = slice(ci * _NCHUNK, (ci + 1) * _NCHUNK)
                for s in range(SPC):
                    t1, t2, t3, t4, tb1, tb2, tb3 = Wt[s]
                    xs = X[s * NF:(s + 1) * NF, sl]
                    p1 = ps.tile([FH, _NCHUNK], f32, tag="p1")
                    nc.tensor.matmul(p1, lhsT=t1, rhs=xs, start=True, stop=True)
                    h1 = work.tile([FH, _NCHUNK], f32, tag="h1")
                    nc.scalar.activation(out=h1, in_=p1, func=relu, bias=tb1, scale=1.0)
                    p2 = ps.tile([FH, _NCHUNK], f32, tag="p2")
                    nc.tensor.matmul(p2, lhsT=t2, rhs=h1, start=True, stop=True)
                    h2 = work.tile([FH, _NCHUNK], f32, tag="h2")
                    nc.scalar.activation(out=h2, in_=p2, func=relu, bias=tb2, scale=1.0)
                    p3 = ps.tile([NF, _NCHUNK], f32, tag="p3")
                    nc.tensor.matmul(p3, lhsT=t3, rhs=h2, start=True, stop=False)
                    nc.tensor.matmul(p3, lhsT=t4, rhs=xs, start=False, stop=True)
                    yt = yp.tile([NF, _NCHUNK], f32, tag="yt")
                    nc.vector.tensor_scalar(
                        out=yt, in0=p3, scalar1=tb3, scalar2=None,
                        op0=mybir.AluOpType.add,
                    )
                    nc.sync.dma_start(out=y_out[s * NF:(s + 1) * NF, sl], in_=yt)
    return nc


def _dyna_device(x, W):
    from concourse.bass_utils import run_bass_kernel_spmd

    if _DEV["nc"] is None:
        _DEV["nc"] = _build_nc()
    w1, b1, w2, b2, w3, b3, ws, bs = W
    in_maps = []
    for c in range(N_CORES):
        s0 = c * SPC
        sl = slice(s0, s0 + SPC)
        in_maps.append(dict(
            x=np.ascontiguousarray(x[sl].reshape(SPC * NF, HW), np.float32),
            w1t=np.ascontiguousarray(w1[sl].transpose(0, 2, 1), np.float32),
            w2t=np.ascontiguousarray(w2[sl].transpose(0, 2, 1), np.float32),
            w3t=np.ascontiguousarray(w3[sl].transpose(0, 2, 1), np.float32),
            wst=np.ascontiguousarray(ws[sl].transpose(0, 2, 1), np.float32),
            b1=np.ascontiguousarray(b1[sl, :, None], np.float32),
            b2=np.ascontiguousarray(b2[sl, :, None], np.float32),
            b3s=np.ascontiguousarray((b3 + bs)[sl, :, None], np.float32),
        ))
    res = run_bass_kernel_spmd(_DEV["nc"], in_maps, list(range(N_CORES)))
    outs = res.results
    y = np.empty((B, NF, HW), np.float32)
    for c in range(N_CORES):
        y[c * SPC:(c + 1) * SPC] = outs[c]["y"].reshape(SPC, NF, HW)
    return y.reshape(B, NF, S, S)


def kernel(lat, seed, sob_pre_w, sob_pre_b, sob_post_w, sob_post_b, dyna_w, dyna_b,
           res_w1, res_b1, res_w2, res_b2, res_wsc, res_bsc, out_w, out_b):
    lat = np.asarray(lat, np.float32)
    seed = np.asarray(seed, np.float32)
    args = [np.asarray(a, np.float32) for a in
            (sob_pre_w, sob_pre_b, sob_post_w, sob_post_b, dyna_w, dyna_b,
             res_w1, res_b1, res_w2, res_b2, res_wsc, res_bsc, out_w, out_b)]
    (sob_pre_w, sob_pre_b, sob_post_w, sob_post_b, dyna_w, dyna_b,
     res_w1, res_b1, res_w2, res_b2, res_wsc, res_bsc, out_w, out_b) = args

    W = _dyna_weights(lat, dyna_w, dyna_b)
    out = np.broadcast_to(seed[0:1], (B,) + seed.shape[1:]).astype(np.float32).copy()
    embs = [out.copy()]
    for _ in range(NC_STEPS):
        g = _conv1x1(out, sob_pre_w, sob_pre_b)
        g = _sin_sobel(g)
        g = _conv1x1(g, sob_post_w, sob_post_b)
        xn = _inst_norm(out * g)
        if _DEV["ok"]:
            try:
                dy = _dyna_device(xn, W)
            except Exception as e:  # device path broken -> numpy fallback
                print("device path failed, falling back to numpy:", repr(e),
                      file=sys.stderr)
                _DEV["ok"] = False
                dy = _dyna_numpy(xn, W)
        else:
            dy = _dyna_numpy(xn, W)
        out = out + 0.1 * dy
        embs.append(out.copy())

    y = np.maximum(_conv1x1(out, res_w1, res_b1), 0.0)
    y = _conv1x1(y, res_w2, res_b2)
    y = y + _conv1x1(out, res_wsc, res_bsc)
    out_raw = _conv1x1(y, out_w, out_b)
    out_img = np.clip(out_raw, -1.0, 1.0)
    out_embs = np.stack(embs, axis=0)
    return out_img, out_embs, out_raw
